# revision 1
# baseline (speedup 1.0000x reference)
"""Locally-connected Conv2d (nn.Conv2dLocal) Trainium2 Bass kernel.

Problem (hardcoded):
  x:      [B=64, C=64, H=32, W=32]  f32
  weight: [OH=32, OW=32, O=64, C=64, KH=3, KW=3] f32
  bias:   [O=64, OH=32, OW=32] f32
  out:    [B=64, O=64, OH=32, OW=32] f32
  out[b,o,oh,ow] = bias[o,oh,ow]
      + sum_{c,kh,kw} x[b,c,oh+kh-1,ow+kw-1] * weight[oh,ow,o,c,kh,kw]

Sharding: 8 cores, core i owns output rows oh in [4i, 4i+4).

Per-core schedule:
  - padded input slab rows r = 0..5 (padded coords), cols iw = 0..33.
  - x strips: strip p = rows (p, p+1), partitions (row, c).
  - per (ow-quarter q, column iw): one weight chunk DMA holding 6 tiles:
      tiles 0..3: kh={0,1} pair for oh=0..3 (K=128 = (kh, c))
      tile  4   : kh=2 for oh=1 (rows 0:64) / oh=0 (rows 64:128)
      tile  5   : kh=2 for oh=2 (rows 0:64) / oh=3 (rows 64:128)
    each tile spans cols (ow asc, o), accumulated into a PSUM bank per
    (q, oh): [64=b, 512=(ow8,o)].
  - bias: rank-1 (ones x bias) bf16 hi/lo matmuls open each bank group.
  - evacuation: ScalarE copy PSUM->SBUF, then DMA out.

Precision MODE:
  "f16x3": x,w split hi+lo fp16; 3 matmuls per logical MM
           (xh*wh + xl*wh + xh*wl) -> ~1e-6 rel err, 1 PE cycle/row.
  "bf16":  single bf16 pass -> ~3e-3 rel err, minimum DMA.
  "fp32":  plain fp32 (4 PE cycles/row).
"""

import numpy as np
import ml_dtypes

B, C, H, W = 64, 64, 32, 32
O, KH, KW = 64, 3, 3
NCORES = 8
RPC = 4              # output rows per core
SLAB = RPC + 2       # padded input rows per core
PW = W + 2           # padded width (34)
NQ = 4               # ow quarters
QW = 8               # ow per quarter
QCOLS = QW + 2       # columns per quarter (10)

MODE = "f16x3"

_cache = {}


def _sched():
    chunks = []
    off = 0
    for q in range(NQ):
        for iw in range(QW * q, QW * q + QCOLS):
            ows = [ow for ow in (iw - 2, iw - 1, iw) if QW * q <= ow < QW * q + QW]
            n = len(ows) * O
            mms = [
                # (oh, strip, p0, psz, tile_idx, tile_p0)
                (0, 0, 0, 128, 0, 0),
                (1, 1, 0, 128, 1, 0),
                (2, 2, 0, 128, 2, 0),
                (3, 3, 0, 128, 3, 0),
                (1, 3, 0, 64, 4, 0),     # kh2: row 3 = strip3 top
                (0, 1, 64, 64, 4, 64),   # kh2: row 2 = strip1 bottom
                (2, 4, 0, 64, 5, 0),     # kh2: row 4 = strip4 top
                (3, 4, 64, 64, 5, 64),   # kh2: row 5 = strip4 bottom
            ]
            chunks.append(dict(q=q, iw=iw, ows=ows, n=n, off=off, mms=mms))
            off += 6 * n
    return chunks, off


def _host_arrays(x, weight, bias):
    """Per-core input dicts, all DMA-contiguous."""
    chunks, total = _sched()
    f16 = ml_dtypes.bfloat16 if MODE == "bf16" else np.float16
    xp = np.pad(x, ((0, 0), (0, 0), (1, 1), (1, 1)))
    in_maps = []
    for i in range(NCORES):
        slab = xp[:, :, RPC * i:RPC * i + SLAB, :]          # [B, C, 6, 34]
        strips = np.stack([
            np.ascontiguousarray(
                slab[:, :, p:p + 2, :].transpose(2, 1, 3, 0).reshape(128, PW * B))
            for p in range(SLAB - 1)
        ])                                                   # [5, 128, 2176] f32
        # per-quarter strip slices (10 columns each), all 5 strips packed
        # into one row of columns: [q, 128, strip * (hi|lo) * 640]
        QC = QCOLS * B                                       # 640
        ns = SLAB - 1
        if MODE == "f16x3":
            xs = np.empty((NQ, 128, ns * 2 * QC), dtype=np.float16)
        elif MODE == "bf16":
            xs = np.empty((NQ, 128, ns * QC), dtype=ml_dtypes.bfloat16)
        else:
            xs = np.empty((NQ, 128, ns * QC), dtype=np.float32)
        for q in range(NQ):
            sl = strips[:, :, QW * q * B:(QW * q + QCOLS) * B]  # [5, 128, 640]
            for s in range(ns):
                if MODE == "f16x3":
                    hi = sl[s].astype(np.float16)
                    lo = (sl[s] - hi.astype(np.float32)).astype(np.float16)
                    xs[q, :, s * 2 * QC:s * 2 * QC + QC] = hi
                    xs[q, :, s * 2 * QC + QC:(s + 1) * 2 * QC] = lo
                else:
                    xs[q, :, s * QC:(s + 1) * QC] = sl[s].astype(xs.dtype)

        w4 = weight[RPC * i:RPC * i + RPC]                   # [4, 32, O, C, 3, 3]
        ws = np.empty((128, total), dtype=np.float32)
        for ch in chunks:
            iw, ows, n, off = ch["iw"], ch["ows"], ch["n"], ch["off"]
            cols = []
            for oh in range(4):                              # tiles 0..3 (kh01)
                blocks = [
                    w4[oh, ow, :, :, 0:2, iw - ow].transpose(2, 1, 0).reshape(128, O)
                    for ow in ows
                ]
                cols.append(np.concatenate(blocks, axis=1))
            for top_oh, bot_oh in ((1, 0), (2, 3)):          # tiles 4, 5 (kh2)
                top = np.concatenate(
                    [w4[top_oh, ow, :, :, 2, iw - ow].T for ow in ows], axis=1)
                bot = np.concatenate(
                    [w4[bot_oh, ow, :, :, 2, iw - ow].T for ow in ows], axis=1)
                cols.append(np.concatenate([top, bot], axis=0))
            ws[:, off:off + 6 * n] = np.concatenate(cols, axis=1)
        if MODE == "f16x3":
            wpk = np.empty((128, 2 * total), dtype=np.float16)
            for ch in chunks:
                n6, off = 6 * ch["n"], ch["off"]
                blk = ws[:, off:off + n6]
                hi = blk.astype(np.float16)
                lo = (blk - hi.astype(np.float32)).astype(np.float16)
                wpk[:, 2 * off:2 * off + n6] = hi
                wpk[:, 2 * off + n6:2 * off + 2 * n6] = lo
        elif MODE == "bf16":
            wpk = ws.astype(ml_dtypes.bfloat16)
        else:
            wpk = ws

        # bias: bf16 hi/lo rows [NQ, 1, 2*2048]
        b4 = bias[:, RPC * i:RPC * i + RPC, :].transpose(1, 2, 0)  # [oh, ow, o]
        bse = np.empty((NQ, 1, 2 * RPC * QW * O), dtype=ml_dtypes.bfloat16)
        for q in range(NQ):
            flat = np.ascontiguousarray(
                b4[:, QW * q:QW * q + QW, :]).reshape(-1)
            hi = flat.astype(ml_dtypes.bfloat16)
            lo = (flat - hi.astype(np.float32)).astype(ml_dtypes.bfloat16)
            bse[q, 0, :flat.size] = hi
            bse[q, 0, flat.size:] = lo
        in_maps.append({"xs": np.ascontiguousarray(xs),
                        "ws": np.ascontiguousarray(wpk), "bse": bse})
    return in_maps


def _build_program():
    from contextlib import ExitStack
    import concourse.bass as bass
    import concourse.bacc as bacc
    import concourse.tile as tile
    from concourse import mybir

    F32 = mybir.dt.float32
    BF16 = mybir.dt.bfloat16
    if MODE == "f16x3":
        WDT, XMUL, WMUL = mybir.dt.float16, 2, 2
    elif MODE == "bf16":
        WDT, XMUL, WMUL = BF16, 1, 1
    else:
        WDT, XMUL, WMUL = F32, 1, 1
    chunks, total = _sched()

    nc = bacc.Bacc("TRN2", target_bir_lowering=False, debug=False,
                   num_devices=NCORES)
    QC = QCOLS * B
    xs_d = nc.dram_tensor("xs", [NQ, 128, (SLAB - 1) * XMUL * QC], WDT,
                          kind="ExternalInput")
    ws_d = nc.dram_tensor("ws", [128, WMUL * total], WDT, kind="ExternalInput")
    bse_d = nc.dram_tensor("bse", [NQ, 1, 2 * RPC * QW * O], BF16,
                           kind="ExternalInput")
    out_d = nc.dram_tensor("out", [B, RPC * W * O], F32, kind="ExternalOutput")

    # stop flag on the last MM per (q, oh) bank group
    laststop = set()
    for q in range(NQ):
        seen = {}
        for ci, ch in enumerate(chunks):
            if ch["q"] != q:
                continue
            for mi, mm in enumerate(ch["mms"]):
                seen.setdefault(mm[0], []).append((ci, mi))
        for oh, lst in seen.items():
            laststop.add(lst[-1])

    with ExitStack() as ctx:
        tc = ctx.enter_context(tile.TileContext(nc))
        xpool = ctx.enter_context(tc.tile_pool(name="xs", bufs=2))
        wpool = ctx.enter_context(tc.tile_pool(name="wt", bufs=4))
        bpool = ctx.enter_context(tc.tile_pool(name="bias", bufs=1))
        opool = ctx.enter_context(tc.tile_pool(name="outs", bufs=2))
        pspool = ctx.enter_context(
            tc.tile_pool(name="ps", bufs=8, space=bass.MemorySpace.PSUM))

        cpool = ctx.enter_context(tc.tile_pool(name="const", bufs=1))
        ones = cpool.tile([1, B], BF16, tag="ones", name="ones")
        nc.gpsimd.memset(ones[:], 1.0)
        NB = 2 * RPC * QW * O  # bias row elems per quarter (hi|lo)
        ball = bpool.tile([1, NQ * NB], BF16, tag="bias", name="bias_all")
        nc.sync.dma_start(ball[:], bse_d.ap().rearrange("q one n -> one (q n)"))

        ws_ap = ws_d.ap()
        out3 = out_d.ap().rearrange("b (oh r) -> b oh r", r=W * O)
        QO = QW * O  # 512, one psum bank
        for q in range(NQ):
            xq = xpool.tile([128, (SLAB - 1) * XMUL * QC], WDT, tag="xq",
                            name=f"xq{q}")
            nc.sync.dma_start(xq[:], xs_d[q])

            def xsl(sp, p0, psz, jl, lo=False):
                base = sp * XMUL * QC + (QC if lo else 0) + jl
                return xq[p0:p0 + psz, base:base + B]

            bt = ball[0:1, q * NB:(q + 1) * NB]
            ps = [pspool.tile([B, QO], F32, tag="psb", name=f"ps{q}_{oh}")
                  for oh in range(RPC)]
            for oh in range(RPC):
                nc.tensor.matmul(ps[oh][:, 0:QO], ones[:],
                                 bt[0:1, oh * QO:(oh + 1) * QO],
                                 start=True, stop=False)
                nc.tensor.matmul(ps[oh][:, 0:QO], ones[:],
                                 bt[0:1, RPC * QO + oh * QO:RPC * QO + (oh + 1) * QO],
                                 start=False, stop=False)
            qchunks = [(ci, ch) for ci, ch in enumerate(chunks) if ch["q"] == q]
            for g in range(0, len(qchunks), 2):              # 2 chunks per DMA
                pair = qchunks[g:g + 2]
                goff = pair[0][1]["off"]
                gcols = sum(6 * ch["n"] for _, ch in pair)
                wt = wpool.tile([128, WMUL * gcols], WDT, tag="wtile",
                                name=f"wt{q}_{g}")
                nc.sync.dma_start(wt[:], ws_ap[:, WMUL * goff:WMUL * (goff + gcols)])
                for ci, ch in pair:
                    iw, ows, n = ch["iw"], ch["ows"], ch["n"]
                    n6 = 6 * n
                    toff = WMUL * (ch["off"] - goff)         # base col in wt
                    c0 = (ows[0] - QW * q) * O
                    jl = (iw - QW * q) * B
                    for mi, mm in enumerate(ch["mms"]):
                        oh, sp, p0, psz, ti, tp0 = mm
                        stop = (ci, mi) in laststop
                        xh = xsl(sp, p0, psz, jl)
                        wh = wt[tp0:tp0 + psz, toff + ti * n:toff + ti * n + n]
                        if MODE == "f16x3":
                            xl = xsl(sp, p0, psz, jl, lo=True)
                            wl = wt[tp0:tp0 + psz,
                                    toff + n6 + ti * n:toff + n6 + ti * n + n]
                            nc.tensor.matmul(ps[oh][:, c0:c0 + n], xh, wh,
                                             start=False, stop=False)
                            nc.tensor.matmul(ps[oh][:, c0:c0 + n], xl, wh,
                                             start=False, stop=False)
                            nc.tensor.matmul(ps[oh][:, c0:c0 + n], xh, wl,
                                             start=False, stop=stop)
                        else:
                            nc.tensor.matmul(ps[oh][:, c0:c0 + n], xh, wh,
                                             start=False, stop=stop)
            ot = opool.tile([B, RPC * QO], F32, tag="ot", name=f"ot{q}")
            for oh in range(RPC):
                nc.scalar.copy(ot[:, oh * QO:(oh + 1) * QO], ps[oh][:])
            nc.sync.dma_start(
                out3[:, :, q * QO:(q + 1) * QO],
                ot[:].rearrange("b (oh r) -> b oh r", r=QO))

    nc.compile()
    return nc


def kernel(x, weight, bias):
    x = np.asarray(x, dtype=np.float32)
    weight = np.asarray(weight, dtype=np.float32)
    bias = np.asarray(bias, dtype=np.float32)

    from concourse.bass_utils import run_bass_kernel_spmd

    if "nc" not in _cache:
        _cache["nc"] = _build_program()
    nc = _cache["nc"]

    in_maps = _host_arrays(x, weight, bias)
    res = run_bass_kernel_spmd(nc, in_maps, list(range(NCORES)))
    out = np.empty((B, O, H, W), dtype=np.float32)
    for i in range(NCORES):
        o_i = res.results[i]["out"].reshape(B, RPC, W, O)   # [b, oh_l, ow, o]
        out[:, :, RPC * i:RPC * i + RPC, :] = o_i.transpose(0, 3, 1, 2)
    return out



# revision 2
# speedup vs baseline: 2.1709x; 2.1709x over previous
"""Locally-connected Conv2d (nn.Conv2dLocal) Trainium2 Bass kernel.

Problem (hardcoded):
  x:      [B=64, C=64, H=32, W=32]  f32
  weight: [OH=32, OW=32, O=64, C=64, KH=3, KW=3] f32
  bias:   [O=64, OH=32, OW=32] f32
  out:    [B=64, O=64, OH=32, OW=32] f32
  out[b,o,oh,ow] = bias[o,oh,ow]
      + sum_{c,kh,kw} x[b,c,oh+kh-1,ow+kw-1] * weight[oh,ow,o,c,kh,kw]

Sharding: 8 cores, core i owns output rows oh in [4i, 4i+4).

Single-pass fp16 design (rel err ~4e-4, gate is 2e-2):
  - x slab padded rows r = 0..5, packed as 3 aligned row-pair strips
    P0=(0,1), P1=(2,3), P2=(4,5); partition = (row_in_pair, c).
    Zero duplication: 1.67 MB/core.
  - per output row oh (local), contract K=576 as one K=128 matmul on a
    full pair + one K=64 matmul on a half pair:
      oh=0: P0 x kh{0,1} + P1-top    x kh2
      oh=1: P1 x kh{1,2} + P0-bottom x kh0
      oh=2: P1 x kh{0,1} + P2-top    x kh2
      oh=3: P2 x kh{1,2} + P1-bottom x kh0
  - per (quarter q, column iw) chunk: 6 weight tiles (4 x K128 + 2 x
    stacked K64 pairs), n = len(ows)*64 streamed cols each, accumulated
    into a PSUM bank per (q, oh): [64=b, 512=(ow8,o)].
  - bias: one fp16 rank-1 (ones x bias) matmul opens each bank.
  - weights prefetched fully: 20 group DMAs (2 chunks each) on the SP
    queue; out stores ride the Activation queue to avoid head-of-line
    blocking; evacuation alternates ScalarE/VectorE, converts to fp16.
"""

import numpy as np

B, C, H, W = 64, 64, 32, 32
O, KH, KW = 64, 3, 3
NCORES = 8
RPC = 4              # output rows per core
SLAB = RPC + 2       # padded input rows per core
PW = W + 2           # padded width (34)
NQ = 4               # ow quarters
QW = 8               # ow per quarter
QCOLS = QW + 2       # columns per quarter (10)
QO = QW * O          # 512 = one psum bank
GRP = 2              # chunks per weight DMA group

F16 = np.float16

# (oh, pair, p0, psz, tile, tile_p0)
MMS = [
    (0, 0, 0, 128, 0, 0),
    (1, 1, 0, 128, 1, 0),
    (2, 1, 0, 128, 2, 0),
    (3, 2, 0, 128, 3, 0),
    (0, 1, 0, 64, 4, 0),     # kh2: slab row 2 = P1 top
    (1, 0, 64, 64, 4, 64),   # kh0: slab row 1 = P0 bottom
    (2, 2, 0, 64, 5, 0),     # kh2: slab row 4 = P2 top
    (3, 1, 64, 64, 5, 64),   # kh0: slab row 3 = P1 bottom
]
KSETS = (0, 1, 0, 1)         # kh slice start for tiles 0..3 (2 wide)

_cache = {}


def _sched():
    chunks = []
    off = 0
    for q in range(NQ):
        for iw in range(QW * q, QW * q + QCOLS):
            ows = [ow for ow in (iw - 2, iw - 1, iw) if QW * q <= ow < QW * q + QW]
            n = len(ows) * O
            chunks.append(dict(q=q, iw=iw, ows=ows, n=n, off=off))
            off += 6 * n
    return chunks, off


def _host_arrays(x, weight, bias):
    """Per-core input dicts, all DMA-contiguous."""
    chunks, total = _sched()
    xp = np.pad(x, ((0, 0), (0, 0), (1, 1), (1, 1)))
    in_maps = []
    for i in range(NCORES):
        slab = xp[:, :, RPC * i:RPC * i + SLAB, :]          # [B, C, 6, 34]
        xs = np.stack([
            slab[:, :, 2 * p:2 * p + 2, :].transpose(2, 1, 3, 0)
            .reshape(128, PW * B)
            for p in range(3)
        ]).astype(F16)                                       # [3, 128, 2176]

        w4 = weight[RPC * i:RPC * i + RPC]                   # [4, 32, O, C, 3, 3]
        ws = np.empty((128, total), dtype=F16)
        for ch in chunks:
            iw, ows, n, off = ch["iw"], ch["ows"], ch["n"], ch["off"]
            cols = []
            for oh in range(4):                              # tiles 0..3 (K128)
                s = KSETS[oh]
                blocks = [
                    w4[oh, ow, :, :, s:s + 2, iw - ow]
                    .transpose(2, 1, 0).reshape(128, O)
                    for ow in ows
                ]
                cols.append(np.concatenate(blocks, axis=1))
            for top_oh, bot_oh in ((0, 1), (2, 3)):          # tiles 4, 5 (K64)
                top = np.concatenate(
                    [w4[top_oh, ow, :, :, 2, iw - ow].T for ow in ows], axis=1)
                bot = np.concatenate(
                    [w4[bot_oh, ow, :, :, 0, iw - ow].T for ow in ows], axis=1)
                cols.append(np.concatenate([top, bot], axis=0))
            ws[:, off:off + 6 * n] = np.concatenate(cols, axis=1)

        b4 = bias[:, RPC * i:RPC * i + RPC, :].transpose(1, 2, 0)  # [oh, ow, o]
        bse = np.empty((NQ, 1, RPC * QO), dtype=F16)
        for q in range(NQ):
            bse[q, 0] = np.ascontiguousarray(
                b4[:, QW * q:QW * q + QW, :]).reshape(-1)
        in_maps.append({"xs": np.ascontiguousarray(xs), "ws": ws, "bse": bse})
    return in_maps


def _build_program():
    from contextlib import ExitStack
    import concourse.bass as bass
    import concourse.bacc as bacc
    import concourse.tile as tile
    from concourse import mybir

    F32 = mybir.dt.float32
    FP16 = mybir.dt.float16
    chunks, total = _sched()
    groups = [chunks[g:g + GRP] for g in range(0, len(chunks), GRP)]

    nc = bacc.Bacc("TRN2", target_bir_lowering=False, debug=False,
                   num_devices=NCORES)
    xs_d = nc.dram_tensor("xs", [3, 128, PW * B], FP16, kind="ExternalInput")
    ws_d = nc.dram_tensor("ws", [128, total], FP16, kind="ExternalInput")
    bse_d = nc.dram_tensor("bse", [NQ, 1, RPC * QO], FP16,
                           kind="ExternalInput")
    out_d = nc.dram_tensor("out", [B, RPC * W * O], FP16,
                           kind="ExternalOutput")

    # stop flag on the last MM per (q, oh) bank
    laststop = set()
    for q in range(NQ):
        seen = {}
        for ci, ch in enumerate(chunks):
            if ch["q"] != q:
                continue
            for mi, mm in enumerate(MMS):
                seen.setdefault(mm[0], []).append((ci, mi))
        for oh, lst in seen.items():
            laststop.add(lst[-1])

    with ExitStack() as ctx:
        tc = ctx.enter_context(tile.TileContext(nc))
        xpool = ctx.enter_context(tc.tile_pool(name="xs", bufs=1))
        wpool = ctx.enter_context(tc.tile_pool(name="wt", bufs=len(groups)))
        bpool = ctx.enter_context(tc.tile_pool(name="bias", bufs=1))
        opool = ctx.enter_context(tc.tile_pool(name="outs", bufs=2))
        pspool = ctx.enter_context(
            tc.tile_pool(name="ps", bufs=8, space=bass.MemorySpace.PSUM))

        cpool = ctx.enter_context(tc.tile_pool(name="const", bufs=1))
        ones = cpool.tile([1, B], FP16, tag="ones", name="ones")
        nc.gpsimd.memset(ones[:], 1.0)
        NB = RPC * QO
        ball = bpool.tile([1, NQ * NB], FP16, tag="bias", name="bias_all")
        nc.scalar.dma_start(ball[:], bse_d.ap().rearrange("q one n -> one (q n)"))

        # x pair strips, then all weight groups, on the SP queue
        xst = []
        for p in range(3):
            xt = xpool.tile([128, PW * B], FP16, tag=f"xp{p}", name=f"xp{p}")
            nc.sync.dma_start(xt[:], xs_d[p])
            xst.append(xt)
        ws_ap = ws_d.ap()
        wts = []
        for gi, grp in enumerate(groups):
            goff = grp[0]["off"]
            gcols = sum(6 * ch["n"] for ch in grp)
            wt = wpool.tile([128, gcols], FP16, tag="wtile", name=f"wt{gi}")
            nc.sync.dma_start(wt[:], ws_ap[:, goff:goff + gcols])
            wts.append(wt)

        out3 = out_d.ap().rearrange("b (oh r) -> b oh r", r=W * O)
        gpq = len(groups) // NQ
        for q in range(NQ):
            bt = ball[0:1, q * NB:(q + 1) * NB]
            ps = [pspool.tile([B, QO], F32, tag="psb", name=f"ps{q}_{oh}")
                  for oh in range(RPC)]
            for oh in range(RPC):
                nc.tensor.matmul(ps[oh][:, 0:QO], ones[:],
                                 bt[0:1, oh * QO:(oh + 1) * QO],
                                 start=True, stop=False)
            for gi in range(gpq * q, gpq * (q + 1)):
                wt = wts[gi]
                goff = groups[gi][0]["off"]
                for ch in groups[gi]:
                    ci = chunks.index(ch)
                    iw, ows, n = ch["iw"], ch["ows"], ch["n"]
                    toff = ch["off"] - goff
                    c0 = (ows[0] - QW * q) * O
                    jl = iw * B
                    for mi, mm in enumerate(MMS):
                        oh, pair, p0, psz, ti, tp0 = mm
                        stop = (ci, mi) in laststop
                        xh = xst[pair][p0:p0 + psz, jl:jl + B]
                        wh = wt[tp0:tp0 + psz,
                                toff + ti * n:toff + ti * n + n]
                        nc.tensor.matmul(ps[oh][:, c0:c0 + n], xh, wh,
                                         start=False, stop=stop)
            ot = opool.tile([B, RPC * QO], FP16, tag="ot", name=f"ot{q}")
            for oh in range(RPC):
                dst = ot[:, oh * QO:(oh + 1) * QO]
                if oh % 2 == 0:
                    nc.scalar.copy(dst, ps[oh][:])
                else:
                    nc.vector.tensor_copy(dst, ps[oh][:])
            nc.scalar.dma_start(
                out3[:, :, q * QO:(q + 1) * QO],
                ot[:].rearrange("b (oh r) -> b oh r", r=QO))

    nc.compile()
    return nc


def kernel(x, weight, bias):
    x = np.asarray(x, dtype=np.float32)
    weight = np.asarray(weight, dtype=np.float32)
    bias = np.asarray(bias, dtype=np.float32)

    from concourse.bass_utils import run_bass_kernel_spmd

    if "nc" not in _cache:
        _cache["nc"] = _build_program()
    nc = _cache["nc"]

    in_maps = _host_arrays(x, weight, bias)
    res = run_bass_kernel_spmd(nc, in_maps, list(range(NCORES)))
    out = np.empty((B, O, H, W), dtype=np.float32)
    for i in range(NCORES):
        o_i = res.results[i]["out"].astype(np.float32)
        o_i = o_i.reshape(B, RPC, W, O)                    # [b, oh_l, ow, o]
        out[:, :, RPC * i:RPC * i + RPC, :] = o_i.transpose(0, 3, 1, 2)
    return out
